# revision 1
# baseline (speedup 1.0000x reference)
"""CLUB loss kernel for 8x TRN2 NeuronCores.

Math: the reference computes, per sample b (L=512 positions, D=64 dims):
  mu     = MLP_mu(x);  logvar = tanh(MLP_lv(x));  iv = exp(-logvar)
  positive[d,l] = -(mu - y)^2 * 0.5 * iv
  negative[d,l] = -mean_j (y[d,j] - mu[d,l])^2 * 0.5 * iv
  loss = mean over (b,l) of sum_d (positive - negative)

The pairwise LxL mean collapses via moments of y over positions:
  mean_j (y_j - mu)^2 = Ey2 - 2*mu*Ey + mu^2
so with yd2 = 2*(y - Ey), ysq = y^2, mu = mu_nb + b2:
  loss = -0.5/(B*L) * sum_{b,d,l} [ ((ysq - Ey2) - mu*yd2) * iv ]
       = -0.5/(B*L) * [ sum(ysq*iv) - sum_d Ey2[d]*sum_l iv[d,l]
                        - sum(mu * (yd2*iv)) ]
sum_l iv comes free from exp's accumulator; the three per-dim accumulator
columns are collapsed on-chip by a single ones/ey2-weighted matmul so the
store is one single-packet DMA (a [64,1] store costs 64 tiny descriptors +
16 lazily-arriving semaphore increments, ~5us observed).

Sharding: data-parallel over batch B=8, one sample per core; host does the
tiny final combine.

Precision/speed: fp32 matmul runs at 4 cycles/col on the PE and fp32r (the
1 cycle/col mode) truncates to ~bf16 operand precision anyway — so all
matmul operands (x, W1, W2, relu outputs) are bf16, which also halves the
dominant DMA traffic. y, biases, PSUM accumulation and all elementwise math
stay fp32. Measured loss error vs the fp32 reference: ~2.8e-4 relative.
"""

import sys

if "/opt/trn_rl_repo" not in sys.path:
    sys.path.insert(0, "/opt/trn_rl_repo")

import numpy as np

B, L = 8, 512
XD, YD, H = 192, 64, 128
NCORES = 8
HC = L // 2
WIC = 640  # w1 pack (512) + w2 pack (128)
YBC = 516  # y (512) + b2mu, b2lv, pad, pad

_CACHE: dict = {}


def build_nc(debug: bool = False):
    import concourse.bass as bass
    import concourse.bacc as bacc
    import concourse.tile as tile
    from concourse import mybir

    f32 = mybir.dt.float32
    bf16 = mybir.dt.bfloat16
    AF = mybir.ActivationFunctionType
    OP = mybir.AluOpType

    nc = bacc.Bacc("TRN2", target_bir_lowering=False, debug=debug)

    # one tensor per DMA queue, packed so each queue moves few, large
    # descriptors (descriptor count, not bytes, limits the queues here)
    wi_d = nc.dram_tensor("wi", [128, WIC], bf16, kind="ExternalInput")
    xa_d = nc.dram_tensor("xa", [128, L], bf16, kind="ExternalInput")
    xb_d = nc.dram_tensor("xb", [64, L], bf16, kind="ExternalInput")
    yb_d = nc.dram_tensor("yb", [64, YBC], f32, kind="ExternalInput")
    b1_d = nc.dram_tensor("b1", [128, 2], f32, kind="ExternalInput")
    acc_d = nc.dram_tensor("acc", [4, 2], f32, kind="ExternalOutput")

    with tile.TileContext(nc) as tc:
        with (
            tc.tile_pool(name="sb", bufs=1) as sb,
            tc.tile_pool(name="ps", bufs=1, space=bass.MemorySpace.PSUM) as ps,
            tc.tile_pool(name="hps", bufs=3, space=bass.MemorySpace.PSUM) as hps,
        ):
            ones = sb.tile([64, 1], f32, tag="ones")
            nc.gpsimd.memset(ones, 1.0)

            # input DMAs: w-pack and xa on their own HWDGE rings (they gate
            # the first matmuls), xb first on SWDGE, then y/biases
            wit = sb.tile([128, WIC], bf16, tag="wit")
            nc.scalar.dma_start(out=wit, in_=wi_d[:, :])
            xat = sb.tile([128, L], bf16, tag="xat")
            nc.sync.dma_start(out=xat, in_=xa_d[:, :])
            xbr = sb.tile([128, L], bf16, tag="xbr")
            nc.gpsimd.dma_start(out=xbr[64:128, :], in_=xb_d[:, :])
            ybt = sb.tile([64, YBC], f32, tag="ybt")
            nc.gpsimd.dma_start(out=ybt, in_=yb_d[:, :])
            b1t = sb.tile([128, 2], f32, tag="b1t")
            nc.gpsimd.dma_start(out=b1t, in_=b1_d[:, :])

            w1lvT_a = wit[:, 0:128]
            w1muT_a = wit[:, 128:256]
            w1lvT_b = wit[64:128, 256:384]
            w1muT_b = wit[64:128, 384:512]
            w2lvT = wit[:, 512:576]
            w2muT = wit[:, 576:640]
            b1mu = b1t[:, 0:1]
            b1lv = b1t[:, 1:2]
            xa = xat[:, :]
            xb = xbr[64:128, :]
            y = ybt[:, 0:512]
            b2mu = ybt[:, 512:513]
            b2lv = ybt[:, 513:514]

            # --- moments of y (DVE, early — overlaps the DMA/matmul wait) ---
            sums = sb.tile([64, 2], f32, tag="sums")
            nc.vector.reduce_sum(out=sums[:, 0:1], in_=y, axis=mybir.AxisListType.X)
            ysq = sb.tile([64, L], f32, tag="ysq")
            nc.vector.scalar_tensor_tensor(
                out=ysq, in0=y, scalar=1.0, in1=y,
                op0=OP.mult, op1=OP.mult, accum_out=sums[:, 1:2],
            )
            eyb = sb.tile([64, 2], f32, tag="eyb")
            nc.vector.tensor_scalar_mul(out=eyb, in0=sums, scalar1=1.0 / L)
            ey = eyb[:, 0:1]
            ey2 = eyb[:, 1:2]
            yd2 = sb.tile([64, L], f32, tag="yd2")
            nc.vector.tensor_scalar(
                out=yd2, in0=y, scalar1=ey, scalar2=2.0, op0=OP.subtract, op1=OP.mult
            )

            # --- MLP, both paths chunked over L halves for pipelining.
            # lv half 0 first (its tail is two ACT stages deeper); explicit
            # ordering edges keep the scheduler from starving the lv tail.
            from concourse.tile import add_dep_helper

            acc6 = sb.tile([64, 6], f32, tag="acc6")
            h_lv_s = sb.tile([128, L], bf16, tag="hlvs")
            h_mu_s = sb.tile([128, L], bf16, tag="hmus")
            mm = {}
            act_order = []
            dve_order = []
            for c in range(2):
                cs = slice(c * HC, (c + 1) * HC)
                # layer 1, lv then mu for this half
                h_lv = hps.tile([128, HC], f32, tag="h")
                mm[f"alv{c}"] = nc.tensor.matmul(
                    h_lv, w1lvT_a, xa[:, cs], start=True, stop=False
                )
                mm[f"blv{c}"] = nc.tensor.matmul(
                    h_lv, w1lvT_b, xb[:, cs], start=False, stop=True
                )
                h_mu = hps.tile([128, HC], f32, tag="h")
                mm[f"amu{c}"] = nc.tensor.matmul(
                    h_mu, w1muT_a, xa[:, cs], start=True, stop=False
                )
                mm[f"bmu{c}"] = nc.tensor.matmul(
                    h_mu, w1muT_b, xb[:, cs], start=False, stop=True
                )
                # relu_lv on ACT, relu_mu on DVE
                act_order.append(
                    nc.scalar.activation(
                        out=h_lv_s[:, cs], in_=h_lv, func=AF.Relu, bias=b1lv, scale=1.0
                    )
                )
                dve_order.append(
                    nc.vector.tensor_scalar(
                        out=h_mu_s[:, cs], in0=h_mu, scalar1=b1mu, scalar2=0.0,
                        op0=OP.add, op1=OP.max,
                    )
                )
                # layer 2
                lv_nb = ps.tile([64, HC], f32, tag=f"lvnb{c}")
                mm[f"w2lv{c}"] = nc.tensor.matmul(
                    lv_nb, w2lvT, h_lv_s[:, cs], start=True, stop=True
                )
                mu_nb = ps.tile([64, HC], f32, tag=f"munb{c}")
                mm[f"w2mu{c}"] = nc.tensor.matmul(
                    mu_nb, w2muT, h_mu_s[:, cs], start=True, stop=True
                )
                # lv tail: tanh -> exp (+free sum(iv) via ACT accumulator)
                t1 = sb.tile([64, HC], f32, tag="t1")
                act_order.append(
                    nc.scalar.activation(
                        out=t1, in_=lv_nb, func=AF.Tanh, bias=b2lv, scale=1.0
                    )
                )
                iv = sb.tile([64, HC], f32, tag="iv")
                if c == 1:
                    # half 1: sum(iv) via ACT's accumulator — keeps the last
                    # DVE chain after exp1 to just finA1+finB1
                    act_order.append(
                        nc.scalar.activation(
                            out=iv, in_=t1, func=AF.Exp, scale=-1.0,
                            accum_out=acc6[:, 4 + c : 5 + c],
                        )
                    )
                else:
                    act_order.append(
                        nc.scalar.activation(out=iv, in_=t1, func=AF.Exp, scale=-1.0)
                    )
                    dve_order.append(
                        nc.vector.reduce_sum(
                            out=acc6[:, 4 + c : 5 + c], in_=iv,
                            axis=mybir.AxisListType.X,
                        )
                    )
                # m2 = (mu_nb + b2mu) * yd2 on DVE
                m2 = sb.tile([64, HC], f32, tag="m2")
                dve_order.append(
                    nc.vector.scalar_tensor_tensor(
                        out=m2, in0=mu_nb, scalar=b2mu, in1=yd2[:, cs],
                        op0=OP.add, op1=OP.mult,
                    )
                )
                # finA1 = sum(ysq * iv), finB = sum(m2 * iv)
                scrA = sb.tile([64, HC], f32, tag="scrA")
                dve_order.append(
                    nc.vector.scalar_tensor_tensor(
                        out=scrA, in0=ysq[:, cs], scalar=1.0, in1=iv,
                        op0=OP.mult, op1=OP.mult, accum_out=acc6[:, 2 * c : 2 * c + 1],
                    )
                )
                scrB = sb.tile([64, HC], f32, tag="scrB")
                dve_order.append(
                    nc.vector.scalar_tensor_tensor(
                        out=scrB, in0=m2, scalar=1.0, in1=iv,
                        op0=OP.mult, op1=OP.mult,
                        accum_out=acc6[:, 2 * c + 1 : 2 * c + 2],
                    )
                )
            # PE stream: half-1 layer-1 matmuls fill the gaps while the relus
            # for half 0 run on ACT/DVE
            pe_order = [
                mm["alv0"], mm["blv0"], mm["amu0"], mm["bmu0"],
                mm["alv1"], mm["w2lv0"], mm["blv1"], mm["w2mu0"],
                mm["amu1"], mm["bmu1"], mm["w2lv1"], mm["w2mu1"],
            ]
            for order in (pe_order, act_order, dve_order):
                for a, b in zip(order[1:], order[:-1]):
                    add_dep_helper(a.ins, b.ins, sync=False, reason="stream-order")
            # two-matmul collapse: the ey2 dot-product over the iv-sums is
            # ready ~1us before the last finB, so it runs early and off the
            # critical path; its PSUM tile reuses a freed h-pool slot.
            civ_ps = hps.tile([2, 1], f32, tag="h")
            mm_civ = nc.tensor.matmul(civ_ps, acc6[:, 4:6], ey2, start=True, stop=True)
            acc_ps = ps.tile([4, 1], f32, tag="accps")
            mm_acc = nc.tensor.matmul(acc_ps, acc6[:, 0:4], ones, start=True, stop=True)
            add_dep_helper(mm_acc.ins, mm_civ.ins, sync=False, reason="civ-first")
            acc_sb = sb.tile([4, 2], f32, tag="accsb")
            nc.vector.tensor_copy(acc_sb[0:2, 1:2], civ_ps)
            nc.vector.tensor_copy(acc_sb[:, 0:1], acc_ps)
            nc.sync.dma_start(out=acc_d[:, :], in_=acc_sb, single_packet=True)

    nc.compile()
    return nc


def pack_inputs(inputs: dict) -> list[dict]:
    import ml_dtypes

    bf = ml_dtypes.bfloat16
    x = np.asarray(inputs["x_samples"], dtype=np.float32)
    y = np.ascontiguousarray(np.asarray(inputs["y_samples"], dtype=np.float32))
    mu_W1 = np.asarray(inputs["mu_W1"], dtype=np.float32)
    mu_b1 = np.asarray(inputs["mu_b1"], dtype=np.float32)
    mu_W2 = np.asarray(inputs["mu_W2"], dtype=np.float32)
    mu_b2 = np.asarray(inputs["mu_b2"], dtype=np.float32)
    lv_W1 = np.asarray(inputs["lv_W1"], dtype=np.float32)
    lv_b1 = np.asarray(inputs["lv_b1"], dtype=np.float32)
    lv_W2 = np.asarray(inputs["lv_W2"], dtype=np.float32)
    lv_b2 = np.asarray(inputs["lv_b2"], dtype=np.float32)

    wi = np.zeros((128, WIC), bf)
    w1muT = mu_W1.T  # [192, 128]
    w1lvT = lv_W1.T
    wi[:, 0:128] = w1lvT[0:128].astype(bf)
    wi[:, 128:256] = w1muT[0:128].astype(bf)
    wi[64:128, 256:384] = w1lvT[128:192].astype(bf)
    wi[64:128, 384:512] = w1muT[128:192].astype(bf)
    wi[:, 512:576] = lv_W2.T.astype(bf)
    wi[:, 576:640] = mu_W2.T.astype(bf)
    b1 = np.ascontiguousarray(np.stack([mu_b1, lv_b1], axis=1))  # [128, 2]

    xb16 = x.astype(bf)
    in_maps = []
    for b in range(NCORES):
        yb = np.zeros((64, YBC), np.float32)
        yb[:, 0:512] = y[b]
        yb[:, 512] = mu_b2
        yb[:, 513] = lv_b2
        in_maps.append(
            {
                "wi": wi,
                "xa": np.ascontiguousarray(xb16[b, 0:128]),
                "xb": np.ascontiguousarray(xb16[b, 128:192]),
                "yb": yb,
                "b1": b1,
            }
        )
    return in_maps


def _combine(results) -> float:
    tot = 0.0
    for r in results:
        a = r["acc"].astype(np.float64)  # [4, 2]
        # col 0: finA1_0, finB_0, finA1_1, finB_1; col 1 rows 0:2: ey2-dot of siv halves
        tot += (a[0, 0] + a[2, 0]) - (a[1, 0] + a[3, 0]) - (a[0, 1] + a[1, 1])
    return tot


def kernel(**inputs) -> np.ndarray:
    from concourse.bass_utils import run_bass_kernel_spmd

    if "nc" not in _CACHE:
        _CACHE["nc"] = build_nc(debug=False)
    nc = _CACHE["nc"]

    in_maps = pack_inputs(inputs)
    res = run_bass_kernel_spmd(nc, in_maps, core_ids=list(range(NCORES)))
    loss = -0.5 * _combine(res.results) / (B * L)
    return np.array(loss, dtype=np.float32)



# revision 5
# speedup vs baseline: 1.0037x; 1.0037x over previous
"""CLUB loss kernel for 8x TRN2 NeuronCores.

Math: the reference computes, per sample b (L=512 positions, D=64 dims):
  mu     = MLP_mu(x);  logvar = tanh(MLP_lv(x));  iv = exp(-logvar)
  loss = mean over (b,l) of sum_d (positive - negative)

The pairwise LxL mean collapses via moments of y over positions:
  pos - neg = -(0.5*iv) * (y^2 - Ey2 - mu*yd2),  yd2 = 2*(y - Ey)
so per core the kernel accumulates one scalar
  acc = sum_{d,l} (A - mu*yd2) * iv,   A = y^2 - Ey2
(the Ey2 term is folded into A, so no separate iv-sum / ey2-dot is
needed) and the host computes  loss = -0.5 * sum_b acc_b / (B*L).

Schedule notes (v2, from perfetto traces of the v1 kernel):
 - ~13.2us of the measured window is framework overhead (preamble const
   memsets + walrus's end-of-NEFF semaphore-reset storm + barriers); the
   body only controls the rest.
 - critical inputs (x, weights, biases) all go on the two HWDGE rings
   (sync + scalar); only y rides SWDGE. v1 had b1 third in the SWDGE
   queue, gating the first relu ~2us late.
 - a chain of dummy matmuls on memset data runs during the input-DMA
   wait so the PE HAM clock-gate (1.2 -> 2.4 GHz after ~3.4us of
   activity) is warm when the real matmuls start.
 - full-L (N=512) matmuls and activations: fewer instructions, and the
   ACT fixed cost (~352 cycles/op) is paid 3x instead of 6x.
 - tail after the last exp is a single DVE accumulate (m3 = A - mu*yd2
   is ready earlier), then a [64,1]x[64,1] matmul collapse and a
   4-byte single-packet store.

Precision: fp32 matmul runs at 4 cycles/col on the PE and fp32r
truncates to ~bf16 anyway - so all matmul operands are bf16 (halves DMA
too). y, biases, PSUM accumulation and elementwise math stay fp32.
"""

import sys

if "/opt/trn_rl_repo" not in sys.path:
    sys.path.insert(0, "/opt/trn_rl_repo")

import numpy as np

B, L = 8, 512
XD, YD, H = 192, 64, 128
NCORES = 8
WIC = 640  # w1 pack (512) + w2 pack (128)
YBC = 516  # y (512) + b2mu, b2lv, pad, pad
NDUMMY = 10  # PE HAM warmup matmuls (N=256 each, ~320ns cold)

_CACHE: dict = {}


def build_nc(debug: bool = False):
    import concourse.bass as bass
    import concourse.bacc as bacc
    import concourse.tile as tile
    from concourse import mybir
    from concourse.tile import add_dep_helper

    f32 = mybir.dt.float32
    bf16 = mybir.dt.bfloat16
    AF = mybir.ActivationFunctionType
    OP = mybir.AluOpType
    AX = mybir.AxisListType

    nc = bacc.Bacc("TRN2", target_bir_lowering=False, debug=debug)

    wi_d = nc.dram_tensor("wi", [128, WIC], bf16, kind="ExternalInput")
    xa_d = nc.dram_tensor("xa", [128, L], bf16, kind="ExternalInput")
    xb_d = nc.dram_tensor("xb", [64, L], bf16, kind="ExternalInput")
    yb_d = nc.dram_tensor("yb", [64, YBC], f32, kind="ExternalInput")
    b1_d = nc.dram_tensor("b1", [128, 2], f32, kind="ExternalInput")
    acc_d = nc.dram_tensor("acc", [1, 1], f32, kind="ExternalOutput")

    with tile.TileContext(nc) as tc:
        with (
            tc.tile_pool(name="sb", bufs=1) as sb,
            tc.tile_pool(name="ps", bufs=1, space=bass.MemorySpace.PSUM) as ps,
        ):
            # --- PE HAM warmup: matmuls on memset data while input DMAs fly
            warm = sb.tile([128, 256], bf16, tag="warm")
            mset_warm = nc.gpsimd.memset(warm, 0.0)
            ones = sb.tile([64, 1], f32, tag="ones")
            nc.gpsimd.memset(ones, 1.0)
            dummy_ps = ps.tile([128, 256], f32, tag="dummy")
            pe_order = []
            for i in range(NDUMMY):
                pe_order.append(
                    nc.tensor.matmul(
                        dummy_ps, warm[:, 0:128], warm[:, 0:256],
                        start=True, stop=True,
                    )
                )

            # --- input DMAs: x/b1 on the sync HWDGE ring, w on the scalar
            # HWDGE ring, y on SWDGE (not latency-critical)
            xat = sb.tile([128, L], bf16, tag="xat")
            d_xa = nc.sync.dma_start(out=xat, in_=xa_d[:, :])
            b1t = sb.tile([128, 2], f32, tag="b1t")
            d_b1 = nc.sync.dma_start(out=b1t, in_=b1_d[:, :])
            add_dep_helper(d_b1.ins, d_xa.ins, sync=False, reason="ring-order")
            wit = sb.tile([128, WIC], bf16, tag="wit")
            nc.scalar.dma_start(out=wit, in_=wi_d[:, :])
            xbr = sb.tile([128, L], bf16, tag="xbr")
            d_xb = nc.gpsimd.dma_start(out=xbr[64:128, :], in_=xb_d[:, :])
            add_dep_helper(d_xb.ins, mset_warm.ins, sync=False, reason="warm-first")
            ybt = sb.tile([64, YBC], f32, tag="ybt")
            d_yb = nc.gpsimd.dma_start(out=ybt, in_=yb_d[:, :])
            add_dep_helper(d_yb.ins, d_xb.ins, sync=False, reason="ring-order")

            w1lvT_a = wit[:, 0:128]
            w1muT_a = wit[:, 128:256]
            w1lvT_b = wit[64:128, 256:384]
            w1muT_b = wit[64:128, 384:512]
            w2lvT = wit[:, 512:576]
            w2muT = wit[:, 576:640]
            b1mu = b1t[:, 0:1]
            b1lv = b1t[:, 1:2]
            xa = xat[:, :]
            xb = xbr[64:128, :]
            y = ybt[:, 0:512]
            b2mu = ybt[:, 512:513]
            b2lv = ybt[:, 513:514]

            # --- y moments (DVE + gpsimd, overlap the DMA/matmul wait) ---
            sums = sb.tile([64, 2], f32, tag="sums")
            dve_order = [
                nc.vector.reduce_sum(out=sums[:, 0:1], in_=y, axis=AX.X)
            ]
            ysq = sb.tile([64, L], f32, tag="ysq")
            dve_order.append(
                nc.vector.scalar_tensor_tensor(
                    out=ysq, in0=y, scalar=1.0, in1=y,
                    op0=OP.mult, op1=OP.mult, accum_out=sums[:, 1:2],
                )
            )
            eyb = sb.tile([64, 2], f32, tag="eyb")
            dve_order.append(
                nc.vector.tensor_scalar_mul(out=eyb, in0=sums, scalar1=1.0 / L)
            )
            ey = eyb[:, 0:1]
            ey2 = eyb[:, 1:2]

            # --- MLP, full-L ---
            h_lv = ps.tile([128, L], f32, tag="hlv")
            h_mu = ps.tile([128, L], f32, tag="hmu")
            pe_order.append(
                nc.tensor.matmul(h_lv, w1lvT_a, xa, start=True, stop=False)
            )
            pe_order.append(
                nc.tensor.matmul(h_lv, w1lvT_b, xb, start=False, stop=True)
            )
            pe_order.append(
                nc.tensor.matmul(h_mu, w1muT_a, xa, start=True, stop=False)
            )
            pe_order.append(
                nc.tensor.matmul(h_mu, w1muT_b, xb, start=False, stop=True)
            )
            h_lv_s = sb.tile([128, L], bf16, tag="hlvs")
            act_order = [
                nc.scalar.activation(
                    out=h_lv_s, in_=h_lv, func=AF.Relu, bias=b1lv, scale=1.0
                )
            ]
            h_mu_s = sb.tile([128, L], bf16, tag="hmus")
            dve_order.append(
                nc.vector.tensor_scalar(
                    out=h_mu_s, in0=h_mu, scalar1=b1mu, scalar2=0.0,
                    op0=OP.add, op1=OP.max,
                )
            )
            # yd2 = 2*(y - Ey), A = y^2 - Ey2 (fill DVE idle behind relu_mu)
            yd2 = sb.tile([64, L], f32, tag="yd2")
            dve_order.append(
                nc.vector.tensor_scalar(
                    out=yd2, in0=y, scalar1=ey, scalar2=2.0,
                    op0=OP.subtract, op1=OP.mult,
                )
            )
            A = sb.tile([64, L], f32, tag="A")
            dve_order.append(
                nc.vector.tensor_scalar(
                    out=A, in0=ysq, scalar1=ey2, scalar2=None, op0=OP.subtract
                )
            )
            lv_nb = ps.tile([64, L], f32, tag="lvnb")
            pe_order.append(
                nc.tensor.matmul(lv_nb, w2lvT, h_lv_s, start=True, stop=True)
            )
            mu_nb = ps.tile([64, L], f32, tag="munb")
            pe_order.append(
                nc.tensor.matmul(mu_nb, w2muT, h_mu_s, start=True, stop=True)
            )
            # lv tail on ACT: tanh -> exp
            t1 = sb.tile([64, L], f32, tag="t1")
            act_order.append(
                nc.scalar.activation(
                    out=t1, in_=lv_nb, func=AF.Tanh, bias=b2lv, scale=1.0
                )
            )
            iv = sb.tile([64, L], f32, tag="iv")
            act_order.append(
                nc.scalar.activation(out=iv, in_=t1, func=AF.Exp, scale=-1.0)
            )
            # mu tail on DVE: m2 = (mu_nb + b2mu)*yd2 ; m3 = A - m2
            m2 = sb.tile([64, L], f32, tag="m2")
            dve_order.append(
                nc.vector.scalar_tensor_tensor(
                    out=m2, in0=mu_nb, scalar=b2mu, in1=yd2,
                    op0=OP.add, op1=OP.mult,
                )
            )
            m3 = sb.tile([64, L], f32, tag="m3")
            dve_order.append(
                nc.vector.scalar_tensor_tensor(
                    out=m3, in0=m2, scalar=-1.0, in1=A,
                    op0=OP.mult, op1=OP.add,
                )
            )
            # f = sum_l m3*iv  (per-d accumulator column)
            accT = sb.tile([64, 1], f32, tag="accT")
            scr = sb.tile([64, L], f32, tag="scr")
            dve_order.append(
                nc.vector.scalar_tensor_tensor(
                    out=scr, in0=m3, scalar=1.0, in1=iv,
                    op0=OP.mult, op1=OP.mult, accum_out=accT,
                )
            )
            # collapse over d with a [64,1]x[64,1] matmul, then 4B store
            acc_ps = ps.tile([1, 1], f32, tag="accps")
            pe_order.append(
                nc.tensor.matmul(acc_ps, accT, ones, start=True, stop=True)
            )
            acc_sb = sb.tile([1, 1], f32, tag="accsb")
            nc.vector.tensor_copy(acc_sb, acc_ps)
            nc.sync.dma_start(out=acc_d[:, :], in_=acc_sb, single_packet=True)

            for order in (pe_order, act_order, dve_order):
                for a, b in zip(order[1:], order[:-1]):
                    add_dep_helper(a.ins, b.ins, sync=False, reason="stream-order")

    nc.compile()
    return nc


def pack_inputs(inputs: dict) -> list[dict]:
    import ml_dtypes

    bf = ml_dtypes.bfloat16
    x = np.asarray(inputs["x_samples"], dtype=np.float32)
    y = np.ascontiguousarray(np.asarray(inputs["y_samples"], dtype=np.float32))
    mu_W1 = np.asarray(inputs["mu_W1"], dtype=np.float32)
    mu_b1 = np.asarray(inputs["mu_b1"], dtype=np.float32)
    mu_W2 = np.asarray(inputs["mu_W2"], dtype=np.float32)
    mu_b2 = np.asarray(inputs["mu_b2"], dtype=np.float32)
    lv_W1 = np.asarray(inputs["lv_W1"], dtype=np.float32)
    lv_b1 = np.asarray(inputs["lv_b1"], dtype=np.float32)
    lv_W2 = np.asarray(inputs["lv_W2"], dtype=np.float32)
    lv_b2 = np.asarray(inputs["lv_b2"], dtype=np.float32)

    wi = np.zeros((128, WIC), bf)
    w1muT = mu_W1.T  # [192, 128]
    w1lvT = lv_W1.T
    wi[:, 0:128] = w1lvT[0:128].astype(bf)
    wi[:, 128:256] = w1muT[0:128].astype(bf)
    wi[64:128, 256:384] = w1lvT[128:192].astype(bf)
    wi[64:128, 384:512] = w1muT[128:192].astype(bf)
    wi[:, 512:576] = lv_W2.T.astype(bf)
    wi[:, 576:640] = mu_W2.T.astype(bf)
    b1 = np.ascontiguousarray(np.stack([mu_b1, lv_b1], axis=1))  # [128, 2]

    xb16 = x.astype(bf)
    in_maps = []
    for b in range(NCORES):
        yb = np.zeros((64, YBC), np.float32)
        yb[:, 0:512] = y[b]
        yb[:, 512] = mu_b2
        yb[:, 513] = lv_b2
        in_maps.append(
            {
                "wi": wi,
                "xa": np.ascontiguousarray(xb16[b, 0:128]),
                "xb": np.ascontiguousarray(xb16[b, 128:192]),
                "yb": yb,
                "b1": b1,
            }
        )
    return in_maps


def _combine(results) -> float:
    tot = 0.0
    for r in results:
        tot += float(r["acc"][0, 0])
    return tot


def kernel(**inputs) -> np.ndarray:
    from concourse.bass_utils import run_bass_kernel_spmd

    if "nc" not in _CACHE:
        _CACHE["nc"] = build_nc(debug=False)
    nc = _CACHE["nc"]

    in_maps = pack_inputs(inputs)
    res = run_bass_kernel_spmd(nc, in_maps, core_ids=list(range(NCORES)))
    loss = -0.5 * _combine(res.results) / (B * L)
    return np.array(loss, dtype=np.float32)


# revision 8
# speedup vs baseline: 1.0280x; 1.0242x over previous
"""CLUB loss kernel for 8x TRN2 NeuronCores.

Math: the reference computes, per sample b (L=512 positions, D=64 dims):
  mu     = MLP_mu(x);  logvar = tanh(MLP_lv(x));  iv = exp(-logvar)
  loss = mean over (b,l) of sum_d (positive - negative)

The pairwise LxL mean collapses via moments of y over positions:
  pos - neg = -(0.5*iv) * (y^2 - Ey2 - mu*yd2),  yd2 = 2*(y - Ey)
so per core the kernel accumulates one scalar
  acc = sum_{d,l} (A - mu*yd2) * iv,   A = y^2 - Ey2
(the Ey2 term is folded into A, so no separate iv-sum / ey2-dot is
needed) and the host computes  loss = -0.5 * sum_b acc_b / (B*L).

Schedule notes (v2, from perfetto traces of the v1 kernel):
 - ~13.2us of the measured window is framework overhead (preamble const
   memsets + walrus's end-of-NEFF semaphore-reset storm + barriers); the
   body only controls the rest.
 - critical inputs (x, weights, biases) all go on the two HWDGE rings
   (sync + scalar); only y rides SWDGE. v1 had b1 third in the SWDGE
   queue, gating the first relu ~2us late.
 - a chain of dummy matmuls on memset data runs during the input-DMA
   wait so the PE HAM clock-gate (1.2 -> 2.4 GHz after ~3.4us of
   activity) is warm when the real matmuls start.
 - full-L (N=512) matmuls and activations: fewer instructions, and the
   ACT fixed cost (~352 cycles/op) is paid 3x instead of 6x.
 - tail after the last exp is a single DVE accumulate (m3 = A - mu*yd2
   is ready earlier), then a [64,1]x[64,1] matmul collapse and a
   4-byte single-packet store.

Precision: fp32 matmul runs at 4 cycles/col on the PE and fp32r
truncates to ~bf16 anyway - so all matmul operands are bf16 (halves DMA
too). y, biases, PSUM accumulation and elementwise math stay fp32.
"""

import sys

if "/opt/trn_rl_repo" not in sys.path:
    sys.path.insert(0, "/opt/trn_rl_repo")

import numpy as np

B, L = 8, 512
XD, YD, H = 192, 64, 128
NCORES = 8
WIC = 640  # w1 pack (512) + w2 pack (128)
YBC = 516  # y (512) + b2mu, b2lv, pad, pad
NDUMMY = 15  # PE HAM warmup matmuls (N=256, ~213ns cold each; HAM needs ~3.4us)

_CACHE: dict = {}


def build_nc(debug: bool = False):
    import concourse.bass as bass
    import concourse.bacc as bacc
    import concourse.tile as tile
    from concourse import mybir
    from concourse.tile import add_dep_helper

    f32 = mybir.dt.float32
    bf16 = mybir.dt.bfloat16
    AF = mybir.ActivationFunctionType
    OP = mybir.AluOpType
    AX = mybir.AxisListType

    nc = bacc.Bacc("TRN2", target_bir_lowering=False, debug=debug)

    wi_d = nc.dram_tensor("wi", [128, WIC], bf16, kind="ExternalInput")
    xa_d = nc.dram_tensor("xa", [128, L], bf16, kind="ExternalInput")
    xb_d = nc.dram_tensor("xb", [64, L], bf16, kind="ExternalInput")
    yb_d = nc.dram_tensor("yb", [64, YBC], f32, kind="ExternalInput")
    b1_d = nc.dram_tensor("b1", [128, 2], f32, kind="ExternalInput")
    acc_d = nc.dram_tensor("acc", [1, 1], f32, kind="ExternalOutput")

    with tile.TileContext(nc) as tc:
        with (
            tc.tile_pool(name="sb", bufs=1) as sb,
            tc.tile_pool(name="ps", bufs=1, space=bass.MemorySpace.PSUM) as ps,
        ):
            # --- PE HAM warmup: matmuls on memset data while input DMAs fly
            warm = sb.tile([128, 256], bf16, tag="warm")
            mset_warm = nc.gpsimd.memset(warm, 0.0)
            ones = sb.tile([64, 1], f32, tag="ones")
            nc.gpsimd.memset(ones, 1.0)
            dummy_ps = ps.tile([128, 256], f32, tag="dummy")
            pe_order = []
            for i in range(NDUMMY):
                pe_order.append(
                    nc.tensor.matmul(
                        dummy_ps, warm[:, 0:128], warm[:, 0:256],
                        start=True, stop=True,
                    )
                )

            # --- input DMAs: x/b1 on the sync HWDGE ring, w on the scalar
            # HWDGE ring, y on SWDGE (not latency-critical)
            xat = sb.tile([128, L], bf16, tag="xat")
            d_xa = nc.sync.dma_start(out=xat, in_=xa_d[:, :])
            b1t = sb.tile([128, 2], f32, tag="b1t")
            d_b1 = nc.sync.dma_start(out=b1t, in_=b1_d[:, :])
            add_dep_helper(d_b1.ins, d_xa.ins, sync=False, reason="ring-order")
            wit = sb.tile([128, WIC], bf16, tag="wit")
            d_wi = nc.scalar.dma_start(out=wit, in_=wi_d[:, :])
            ybt = sb.tile([64, YBC], f32, tag="ybt")
            d_yb = nc.scalar.dma_start(out=ybt, in_=yb_d[:, :])
            add_dep_helper(d_yb.ins, d_wi.ins, sync=False, reason="ring-order")
            xbr = sb.tile([128, L], bf16, tag="xbr")
            d_xb = nc.gpsimd.dma_start(out=xbr[64:128, :], in_=xb_d[:, :])
            add_dep_helper(d_xb.ins, mset_warm.ins, sync=False, reason="warm-first")

            w1lvT_a = wit[:, 0:128]
            w1muT_a = wit[:, 128:256]
            w1lvT_b = wit[64:128, 256:384]
            w1muT_b = wit[64:128, 384:512]
            w2lvT = wit[:, 512:576]
            w2muT = wit[:, 576:640]
            b1mu = b1t[:, 0:1]
            b1lv = b1t[:, 1:2]
            xa = xat[:, :]
            xb = xbr[64:128, :]
            y = ybt[:, 0:512]
            b2mu = ybt[:, 512:513]
            b2lv = ybt[:, 513:514]

            # --- y moments (DVE + gpsimd, overlap the DMA/matmul wait) ---
            sums = sb.tile([64, 2], f32, tag="sums")
            dve_order = [
                nc.vector.reduce_sum(out=sums[:, 0:1], in_=y, axis=AX.X)
            ]
            ysq = sb.tile([64, L], f32, tag="ysq")
            dve_order.append(
                nc.vector.scalar_tensor_tensor(
                    out=ysq, in0=y, scalar=1.0, in1=y,
                    op0=OP.mult, op1=OP.mult, accum_out=sums[:, 1:2],
                )
            )
            eyb = sb.tile([64, 2], f32, tag="eyb")
            dve_order.append(
                nc.vector.tensor_scalar_mul(out=eyb, in0=sums, scalar1=1.0 / L)
            )
            ey = eyb[:, 0:1]
            ey2 = eyb[:, 1:2]

            # --- MLP, full-L ---
            h_lv = ps.tile([128, L], f32, tag="hlv")
            h_mu = ps.tile([128, L], f32, tag="hmu")
            pe_order.append(
                nc.tensor.matmul(h_lv, w1lvT_a, xa, start=True, stop=False)
            )
            pe_order.append(
                nc.tensor.matmul(h_lv, w1lvT_b, xb, start=False, stop=True)
            )
            pe_order.append(
                nc.tensor.matmul(h_mu, w1muT_a, xa, start=True, stop=False)
            )
            pe_order.append(
                nc.tensor.matmul(h_mu, w1muT_b, xb, start=False, stop=True)
            )
            h_lv_s = sb.tile([128, L], bf16, tag="hlvs")
            act_order = [
                nc.scalar.activation(
                    out=h_lv_s, in_=h_lv, func=AF.Relu, bias=b1lv, scale=1.0
                )
            ]
            h_mu_s = sb.tile([128, L], bf16, tag="hmus")
            act_order.append(
                nc.scalar.activation(
                    out=h_mu_s, in_=h_mu, func=AF.Relu, bias=b1mu, scale=1.0
                )
            )
            # yd2 = 2*(y - Ey), A = y^2 - Ey2 (DVE, overlap the mu/lv chains)
            yd2 = sb.tile([64, L], f32, tag="yd2")
            dve_order.append(
                nc.vector.tensor_scalar(
                    out=yd2, in0=y, scalar1=ey, scalar2=2.0,
                    op0=OP.subtract, op1=OP.mult,
                )
            )
            A = sb.tile([64, L], f32, tag="A")
            dve_order.append(
                nc.vector.tensor_scalar(
                    out=A, in0=ysq, scalar1=ey2, scalar2=None, op0=OP.subtract
                )
            )
            lv_nb = ps.tile([64, L], f32, tag="lvnb")
            pe_order.append(
                nc.tensor.matmul(lv_nb, w2lvT, h_lv_s, start=True, stop=True)
            )
            mu_nb = ps.tile([64, L], f32, tag="munb")
            pe_order.append(
                nc.tensor.matmul(mu_nb, w2muT, h_mu_s, start=True, stop=True)
            )
            # lv tail on ACT: tanh -> exp
            t1 = sb.tile([64, L], f32, tag="t1")
            act_order.append(
                nc.scalar.activation(
                    out=t1, in_=lv_nb, func=AF.Tanh, bias=b2lv, scale=1.0
                )
            )
            iv = sb.tile([64, L], f32, tag="iv")
            act_order.append(
                nc.scalar.activation(out=iv, in_=t1, func=AF.Exp, scale=-1.0)
            )
            # mu tail on DVE: m2 = (mu_nb + b2mu)*yd2 ; m3 = A - m2
            m2 = sb.tile([64, L], f32, tag="m2")
            dve_order.append(
                nc.vector.scalar_tensor_tensor(
                    out=m2, in0=mu_nb, scalar=b2mu, in1=yd2,
                    op0=OP.add, op1=OP.mult,
                )
            )
            m3 = sb.tile([64, L], f32, tag="m3")
            dve_order.append(
                nc.vector.scalar_tensor_tensor(
                    out=m3, in0=m2, scalar=-1.0, in1=A,
                    op0=OP.mult, op1=OP.add,
                )
            )
            # f = sum_l m3*iv  (per-d accumulator column)
            accT = sb.tile([64, 1], f32, tag="accT")
            scr = sb.tile([64, L], f32, tag="scr")
            dve_order.append(
                nc.vector.scalar_tensor_tensor(
                    out=scr, in0=m3, scalar=1.0, in1=iv,
                    op0=OP.mult, op1=OP.mult, accum_out=accT,
                )
            )
            # collapse over d with a [64,1]x[64,1] matmul, then 4B store
            acc_ps = ps.tile([1, 1], f32, tag="accps")
            pe_order.append(
                nc.tensor.matmul(acc_ps, accT, ones, start=True, stop=True)
            )
            acc_sb = sb.tile([1, 1], f32, tag="accsb")
            nc.vector.tensor_copy(acc_sb, acc_ps)
            nc.sync.dma_start(out=acc_d[:, :], in_=acc_sb, single_packet=True)

            for order in (pe_order, act_order, dve_order):
                for a, b in zip(order[1:], order[:-1]):
                    add_dep_helper(a.ins, b.ins, sync=False, reason="stream-order")

    nc.compile()
    return nc


def pack_inputs(inputs: dict) -> list[dict]:
    import ml_dtypes

    bf = ml_dtypes.bfloat16
    x = np.asarray(inputs["x_samples"], dtype=np.float32)
    y = np.ascontiguousarray(np.asarray(inputs["y_samples"], dtype=np.float32))
    mu_W1 = np.asarray(inputs["mu_W1"], dtype=np.float32)
    mu_b1 = np.asarray(inputs["mu_b1"], dtype=np.float32)
    mu_W2 = np.asarray(inputs["mu_W2"], dtype=np.float32)
    mu_b2 = np.asarray(inputs["mu_b2"], dtype=np.float32)
    lv_W1 = np.asarray(inputs["lv_W1"], dtype=np.float32)
    lv_b1 = np.asarray(inputs["lv_b1"], dtype=np.float32)
    lv_W2 = np.asarray(inputs["lv_W2"], dtype=np.float32)
    lv_b2 = np.asarray(inputs["lv_b2"], dtype=np.float32)

    wi = np.zeros((128, WIC), bf)
    w1muT = mu_W1.T  # [192, 128]
    w1lvT = lv_W1.T
    wi[:, 0:128] = w1lvT[0:128].astype(bf)
    wi[:, 128:256] = w1muT[0:128].astype(bf)
    wi[64:128, 256:384] = w1lvT[128:192].astype(bf)
    wi[64:128, 384:512] = w1muT[128:192].astype(bf)
    wi[:, 512:576] = lv_W2.T.astype(bf)
    wi[:, 576:640] = mu_W2.T.astype(bf)
    b1 = np.ascontiguousarray(np.stack([mu_b1, lv_b1], axis=1))  # [128, 2]

    xb16 = x.astype(bf)
    in_maps = []
    for b in range(NCORES):
        yb = np.zeros((64, YBC), np.float32)
        yb[:, 0:512] = y[b]
        yb[:, 512] = mu_b2
        yb[:, 513] = lv_b2
        in_maps.append(
            {
                "wi": wi,
                "xa": np.ascontiguousarray(xb16[b, 0:128]),
                "xb": np.ascontiguousarray(xb16[b, 128:192]),
                "yb": yb,
                "b1": b1,
            }
        )
    return in_maps


def _combine(results) -> float:
    tot = 0.0
    for r in results:
        tot += float(r["acc"][0, 0])
    return tot


def kernel(**inputs) -> np.ndarray:
    from concourse.bass_utils import run_bass_kernel_spmd

    if "nc" not in _CACHE:
        _CACHE["nc"] = build_nc(debug=False)
    nc = _CACHE["nc"]

    in_maps = pack_inputs(inputs)
    res = run_bass_kernel_spmd(nc, in_maps, core_ids=list(range(NCORES)))
    loss = -0.5 * _combine(res.results) / (B * L)
    return np.array(loss, dtype=np.float32)


# revision 9
# speedup vs baseline: 1.0847x; 1.0552x over previous
"""CLUB loss kernel for 8x TRN2 NeuronCores.

Math: per sample b (L=512 positions, D=64 dims):
  mu     = MLP_mu(x);  logvar = tanh(MLP_lv(x));  iv = exp(-logvar)
  loss = mean over (b,l) of sum_d (positive - negative)
The pairwise LxL mean collapses via moments of y:
  pos - neg = -(0.5*iv) * (y^2 - Ey2 - mu*yd2),  yd2 = 2*(y - Ey)
so per core:  acc = sum_{d,l} (A - mu*yd2) * iv,  A = y^2 - Ey2,
and the host computes  loss = -0.5 * sum_b acc_b / (B*L).

Schedule (v4, built from perfetto traces):
 - ~13.2us of the measured window is fixed framework overhead (bass
   preamble + NRT's load-time epilogue that resets all 253 semaphores
   one EVENT_SEMAPHORE at a time + barrier butterflies). Body work sits
   on top of that.
 - input DMA placement: wi alone on the scalar HWDGE ring (sharing it
   serialized the completion sem ~2.3us late in v3); xa -> yb -> b1 on
   the sync HWDGE ring; xb on SWDGE (first in queue). HWDGE data is
   consumable ~3.2us after desc-gen (SDMA start + transfer + receipt).
 - PE HAM warmup: 6 K=1 matmuls on a memset [1,640] tile keep the PE
   busy from queue-start until the real matmuls begin, with ~zero SBUF
   read traffic (v2's K=128 dummies at ~450GB/s starved the input-DMA
   writes). HAM flips 1.2->2.4GHz after ~3.4us of sustained activity.
 - ACT runs relu_lv, relu_mu, tanh, exp (full-L each, (N+352)/1.2 ns);
   DVE runs the y-moments and the m2 -> m3 -> f tail; y and all
   elementwise intermediates are fp16 for the DVE 2x perf mode.
 - tail: f accumulates sum_l m3*iv per d; a [64,1]x[64,1] matmul
   collapses d; 4-byte single-packet store.

Precision: matmul operands bf16 (fp32 PE mode is 4x slower and fp32r
truncates to ~bf16 anyway); y/intermediates fp16 (loss error vs fp32
reference measured ~3e-4 with fp32, fp16 adds ~5e-4-level noise, well
under the 2e-2 gate); biases and accumulators fp32.
"""

import sys

if "/opt/trn_rl_repo" not in sys.path:
    sys.path.insert(0, "/opt/trn_rl_repo")

import numpy as np

B, L = 8, 512
XD, YD, H = 192, 64, 128
NCORES = 8
WIC = 640  # w1 pack (512) + w2 pack (128)
NDUMMY = 6  # K=1 N=512 HAM-warmup matmuls, ~427ns each cold

_CACHE: dict = {}


def build_nc(debug: bool = False):
    import concourse.bass as bass
    import concourse.bacc as bacc
    import concourse.tile as tile
    from concourse import mybir
    from concourse.tile import add_dep_helper

    f32 = mybir.dt.float32
    f16 = mybir.dt.float16
    bf16 = mybir.dt.bfloat16
    AF = mybir.ActivationFunctionType
    OP = mybir.AluOpType
    AX = mybir.AxisListType

    nc = bacc.Bacc("TRN2", target_bir_lowering=False, debug=debug)

    wi_d = nc.dram_tensor("wi", [128, WIC], bf16, kind="ExternalInput")
    xa_d = nc.dram_tensor("xa", [128, L], bf16, kind="ExternalInput")
    xb_d = nc.dram_tensor("xb", [64, L], bf16, kind="ExternalInput")
    yb_d = nc.dram_tensor("yb", [64, L], f16, kind="ExternalInput")
    b1_d = nc.dram_tensor("b1", [128, 4], f32, kind="ExternalInput")
    acc_d = nc.dram_tensor("acc", [1, 1], f32, kind="ExternalOutput")

    with tile.TileContext(nc) as tc:
        with (
            tc.tile_pool(name="sb", bufs=1) as sb,
            tc.tile_pool(name="ps", bufs=1, space=bass.MemorySpace.PSUM) as ps,
        ):
            # --- PE HAM warmup: K=1 matmuls on memset data (no SBUF load)
            warm = sb.tile([1, WIC], bf16, tag="warm")
            mset_warm = nc.gpsimd.memset(warm, 0.0)
            ones = sb.tile([64, 1], f32, tag="ones")
            nc.gpsimd.memset(ones, 1.0)
            dummy_ps = ps.tile([128, L], f32, tag="dummy")
            pe_order = []
            for i in range(NDUMMY):
                pe_order.append(
                    nc.tensor.matmul(
                        dummy_ps, warm[0:1, 0:128], warm[0:1, 0:512],
                        start=True, stop=True,
                    )
                )

            # --- input DMAs ---
            xat = sb.tile([128, L], bf16, tag="xat")
            d_xa = nc.sync.dma_start(out=xat, in_=xa_d[:, :])
            ybt = sb.tile([64, L], f16, tag="ybt")
            d_yb = nc.sync.dma_start(out=ybt, in_=yb_d[:, :])
            b1t = sb.tile([128, 4], f32, tag="b1t")
            d_b1 = nc.sync.dma_start(out=b1t, in_=b1_d[:, :])
            add_dep_helper(d_yb.ins, d_xa.ins, sync=False, reason="ring-order")
            add_dep_helper(d_b1.ins, d_yb.ins, sync=False, reason="ring-order")
            wit = sb.tile([128, WIC], bf16, tag="wit")
            nc.scalar.dma_start(out=wit, in_=wi_d[:, :])
            xbr = sb.tile([128, L], bf16, tag="xbr")
            d_xb = nc.gpsimd.dma_start(out=xbr[64:128, :], in_=xb_d[:, :])
            add_dep_helper(d_xb.ins, mset_warm.ins, sync=False, reason="warm-first")

            w1lvT_a = wit[:, 0:128]
            w1muT_a = wit[:, 128:256]
            w1lvT_b = wit[64:128, 256:384]
            w1muT_b = wit[64:128, 384:512]
            w2lvT = wit[:, 512:576]
            w2muT = wit[:, 576:640]
            b1mu = b1t[:, 0:1]
            b1lv = b1t[:, 1:2]
            b2mu = b1t[0:64, 2:3]
            b2lv = b1t[0:64, 3:4]
            xa = xat[:, :]
            xb = xbr[64:128, :]
            y = ybt[:, :]

            # --- y moments on DVE (fp16 -> 2x mode), overlap the MLP ---
            sums = sb.tile([64, 2], f32, tag="sums")
            dve_order = [
                nc.vector.reduce_sum(out=sums[:, 0:1], in_=y, axis=AX.X)
            ]
            ysq = sb.tile([64, L], f16, tag="ysq")
            dve_order.append(
                nc.vector.scalar_tensor_tensor(
                    out=ysq, in0=y, scalar=1.0, in1=y,
                    op0=OP.mult, op1=OP.mult, accum_out=sums[:, 1:2],
                )
            )
            eyb = sb.tile([64, 2], f32, tag="eyb")
            dve_order.append(
                nc.vector.tensor_scalar_mul(out=eyb, in0=sums, scalar1=1.0 / L)
            )
            ey = eyb[:, 0:1]
            ey2 = eyb[:, 1:2]
            yd2 = sb.tile([64, L], f16, tag="yd2")
            dve_order.append(
                nc.vector.tensor_scalar(
                    out=yd2, in0=y, scalar1=ey, scalar2=2.0,
                    op0=OP.subtract, op1=OP.mult,
                )
            )
            A = sb.tile([64, L], f16, tag="A")
            dve_order.append(
                nc.vector.tensor_scalar(
                    out=A, in0=ysq, scalar1=ey2, scalar2=None, op0=OP.subtract
                )
            )

            # --- MLP, full-L ---
            h_lv = ps.tile([128, L], f32, tag="hlv")
            h_mu = ps.tile([128, L], f32, tag="hmu")
            pe_order.append(
                nc.tensor.matmul(h_lv, w1lvT_a, xa, start=True, stop=False)
            )
            pe_order.append(
                nc.tensor.matmul(h_mu, w1muT_a, xa, start=True, stop=False)
            )
            pe_order.append(
                nc.tensor.matmul(h_lv, w1lvT_b, xb, start=False, stop=True)
            )
            pe_order.append(
                nc.tensor.matmul(h_mu, w1muT_b, xb, start=False, stop=True)
            )
            h_lv_s = sb.tile([128, L], bf16, tag="hlvs")
            act_order = [
                nc.scalar.activation(
                    out=h_lv_s, in_=h_lv, func=AF.Relu, bias=b1lv, scale=1.0
                )
            ]
            h_mu_s = sb.tile([128, L], bf16, tag="hmus")
            act_order.append(
                nc.scalar.activation(
                    out=h_mu_s, in_=h_mu, func=AF.Relu, bias=b1mu, scale=1.0
                )
            )
            lv_nb = ps.tile([64, L], f32, tag="lvnb")
            pe_order.append(
                nc.tensor.matmul(lv_nb, w2lvT, h_lv_s, start=True, stop=True)
            )
            mu_nb = ps.tile([64, L], f32, tag="munb")
            pe_order.append(
                nc.tensor.matmul(mu_nb, w2muT, h_mu_s, start=True, stop=True)
            )
            # lv tail on ACT: tanh -> exp
            t1 = sb.tile([64, L], f16, tag="t1")
            act_order.append(
                nc.scalar.activation(
                    out=t1, in_=lv_nb, func=AF.Tanh, bias=b2lv, scale=1.0
                )
            )
            iv = sb.tile([64, L], f16, tag="iv")
            act_order.append(
                nc.scalar.activation(out=iv, in_=t1, func=AF.Exp, scale=-1.0)
            )
            # mu tail on DVE: m2 = (mu_nb + b2mu)*yd2 ; m3 = A - m2 ; f = m3*iv
            m2 = sb.tile([64, L], f16, tag="m2")
            dve_order.append(
                nc.vector.scalar_tensor_tensor(
                    out=m2, in0=mu_nb, scalar=b2mu, in1=yd2,
                    op0=OP.add, op1=OP.mult,
                )
            )
            m3 = sb.tile([64, L], f16, tag="m3")
            dve_order.append(
                nc.vector.scalar_tensor_tensor(
                    out=m3, in0=m2, scalar=-1.0, in1=A,
                    op0=OP.mult, op1=OP.add,
                )
            )
            accT = sb.tile([64, 1], f32, tag="accT")
            scr = sb.tile([64, L], f16, tag="scr")
            dve_order.append(
                nc.vector.scalar_tensor_tensor(
                    out=scr, in0=m3, scalar=1.0, in1=iv,
                    op0=OP.mult, op1=OP.mult, accum_out=accT,
                )
            )
            # collapse over d, then 4B store
            acc_ps = ps.tile([1, 1], f32, tag="accps")
            pe_order.append(
                nc.tensor.matmul(acc_ps, accT, ones, start=True, stop=True)
            )
            acc_sb = sb.tile([1, 1], f32, tag="accsb")
            nc.vector.tensor_copy(acc_sb, acc_ps)
            nc.sync.dma_start(out=acc_d[:, :], in_=acc_sb, single_packet=True)

            for order in (pe_order, act_order, dve_order):
                for a, b in zip(order[1:], order[:-1]):
                    add_dep_helper(a.ins, b.ins, sync=False, reason="stream-order")

    nc.compile()
    return nc


def pack_inputs(inputs: dict) -> list[dict]:
    import ml_dtypes

    bf = ml_dtypes.bfloat16
    x = np.asarray(inputs["x_samples"], dtype=np.float32)
    y = np.ascontiguousarray(np.asarray(inputs["y_samples"], dtype=np.float32))
    mu_W1 = np.asarray(inputs["mu_W1"], dtype=np.float32)
    mu_b1 = np.asarray(inputs["mu_b1"], dtype=np.float32)
    mu_W2 = np.asarray(inputs["mu_W2"], dtype=np.float32)
    mu_b2 = np.asarray(inputs["mu_b2"], dtype=np.float32)
    lv_W1 = np.asarray(inputs["lv_W1"], dtype=np.float32)
    lv_b1 = np.asarray(inputs["lv_b1"], dtype=np.float32)
    lv_W2 = np.asarray(inputs["lv_W2"], dtype=np.float32)
    lv_b2 = np.asarray(inputs["lv_b2"], dtype=np.float32)

    wi = np.zeros((128, WIC), bf)
    w1muT = mu_W1.T  # [192, 128]
    w1lvT = lv_W1.T
    wi[:, 0:128] = w1lvT[0:128].astype(bf)
    wi[:, 128:256] = w1muT[0:128].astype(bf)
    wi[64:128, 256:384] = w1lvT[128:192].astype(bf)
    wi[64:128, 384:512] = w1muT[128:192].astype(bf)
    wi[:, 512:576] = lv_W2.T.astype(bf)
    wi[:, 576:640] = mu_W2.T.astype(bf)
    b1 = np.zeros((128, 4), np.float32)
    b1[:, 0] = mu_b1
    b1[:, 1] = lv_b1
    b1[0:64, 2] = mu_b2
    b1[0:64, 3] = lv_b2

    xb16 = x.astype(bf)
    y16 = y.astype(np.float16)
    in_maps = []
    for b in range(NCORES):
        in_maps.append(
            {
                "wi": wi,
                "xa": np.ascontiguousarray(xb16[b, 0:128]),
                "xb": np.ascontiguousarray(xb16[b, 128:192]),
                "yb": np.ascontiguousarray(y16[b]),
                "b1": b1,
            }
        )
    return in_maps


def _combine(results) -> float:
    tot = 0.0
    for r in results:
        tot += float(r["acc"][0, 0])
    return tot


def kernel(**inputs) -> np.ndarray:
    from concourse.bass_utils import run_bass_kernel_spmd

    if "nc" not in _CACHE:
        _CACHE["nc"] = build_nc(debug=False)
    nc = _CACHE["nc"]

    in_maps = pack_inputs(inputs)
    res = run_bass_kernel_spmd(nc, in_maps, core_ids=list(range(NCORES)))
    loss = -0.5 * _combine(res.results) / (B * L)
    return np.array(loss, dtype=np.float32)
